# revision 32
# baseline (speedup 1.0000x reference)
import os
import numpy as np

# nn_LinearPFN on 8 NeuronCores: 2 batches x 4-core groups, row-sharded
# transformer with per-layer AllGather of the ctx rows of the (transposed)
# residual stream. Globally time-sorted keys with statically trimmed causal
# ranges, recency bias folded into V (uniform alpha), 4-head-contiguous K/Q
# tiles with single-op bias adds, bf16 probability/mask datapath (2x DVE),
# batched ln/exp softmax normalization on the scalar engine, PE-broadcast
# LN statistics, last-layer query-only trim.
B, C, Q, S, D = 2, 1536, 512, 2048, 256
NH, DH, DFF, NL = 8, 32, 1024, 6
LIN, HOUT = 20, 10
R = 512            # rows per core: 384 ctx (sorted, r::4) + 128 qry (sorted)
SK = 1536          # ctx-only key columns, globally time-sorted
NSEC = 12          # 128-key sections
NL_RUN = int(os.environ.get("PFN_NL", NL))
Q0_LAST = 256      # last layer computes query cols [256:512) only


def _host_prep(inputs):
    import ml_dtypes
    BF = ml_dtypes.bfloat16
    ctx_x = np.asarray(inputs["ctx_x"], np.float32)
    ctx_z = np.asarray(inputs["ctx_z"], np.float32)
    qry_x = np.asarray(inputs["qry_x"], np.float32)
    t_ctx = np.asarray(inputs["t_ctx"]).astype(np.int64)
    t_qry = np.asarray(inputs["t_qry"]).astype(np.int64)
    alpha = np.asarray(inputs["alpha"], np.float32)
    assert np.allclose(alpha, alpha[0]), "V-fold requires uniform alpha"
    al = float(alpha[0])

    meta = {}
    oc = np.argsort(t_ctx, axis=1, kind="stable")            # [B, C]
    oq = np.argsort(t_qry, axis=1, kind="stable")            # [B, Q]
    tc_sorted = np.take_along_axis(t_ctx, oc, axis=1)        # [B, C]
    tq_sorted = np.take_along_axis(t_qry, oq, axis=1)        # [B, Q]

    # embed features [B, S, 52]: [ctx_x|ctx_z|qry_x|is_ctx|is_qry]
    X = np.zeros((B, S, 52), np.float32)
    X[:, :C, 0:20] = ctx_x
    X[:, :C, 20:30] = ctx_z
    X[:, :C, 50] = 1.0
    X[:, C:, 30:50] = qry_x
    X[:, C:, 51] = 1.0
    Wcat = np.zeros((52, D), np.float32)
    W_ctx = np.asarray(inputs["W_ctx"], np.float32)   # [D, 30]
    W_qry = np.asarray(inputs["W_qry"], np.float32)
    Wcat[0:30] = W_ctx.T
    Wcat[30:50] = W_qry[:, :20].T
    Wcat[50] = np.asarray(inputs["b_ctx"], np.float32)
    Wcat[51] = np.asarray(inputs["b_qry"], np.float32)

    # row times per core r: cols 0:384 ctx-sorted (r::4), 384:512 qry-sorted
    trow_all = np.zeros((B, 4, R), np.float32)
    for r in range(4):
        trow_all[:, r, 0:384] = tc_sorted[:, r::4]
        trow_all[:, r, 384:512] = tq_sorted[:, r::4]

    # static first-allowed query col per key section: qlo[s] = min over
    # (b, core) of the first col whose time >= the section's min key time.
    # Capped at 256 so f32r matmuls keep N>=256 (cpr 1).
    qlo = np.zeros(NSEC, np.int32)
    for s in range(NSEC):
        lo = R
        for b in range(B):
            tmin = tc_sorted[b, 128 * s]
            for r in range(4):
                tr = trow_all[b, r]
                cf = int(np.argmax(tr[0:384] >= tmin)) \
                    if (tr[0:384] >= tmin).any() else 384
                qf = (384 + int(np.argmax(tr[384:] >= tmin))) \
                    if (tr[384:] >= tmin).any() else R
                lo = min(lo, cf if cf < 384 else qf)
        qlo[s] = min(lo, 256) & ~1
    meta["qlo"] = qlo

    # per-key e^{alpha*t} (recency bias folded into V); [128, NSEC] per batch
    vexp = np.zeros((B, 128, NSEC), np.float32)
    for b in range(B):
        for s in range(NSEC):
            vexp[b, :, s] = np.exp(al * tc_sorted[b, 128 * s:128 * (s + 1)]
                                   .astype(np.float32))

    Wi = {n: np.asarray(inputs[n], np.float32) for n in
          ("Wqkv", "bqkv", "Wo", "bo", "ln1_s", "ln1_b", "W1", "b1", "W2", "b2",
           "ln2_s", "ln2_b", "W_out", "b_out")}
    sc = np.float32(1.0 / np.sqrt(DH))
    WqT = Wi["Wqkv"][:, 0:D, :].transpose(0, 2, 1) * sc
    WkT = Wi["Wqkv"][:, D:2 * D, :].transpose(0, 2, 1)
    WvT = Wi["Wqkv"][:, 2 * D:, :].transpose(0, 2, 1)
    WoT = Wi["Wo"].transpose(0, 2, 1)
    W1T = Wi["W1"].transpose(0, 2, 1)      # [NL, D, DFF]
    W2T = Wi["W2"].transpose(0, 2, 1)      # [NL, DFF, D]

    # packed attention weights: col block 256*(2*w + d), w in (q,k,v,o)
    Wpack = np.zeros((NL, 128, 2048), np.float32)
    for w, WT in enumerate((WqT, WkT, WvT, WoT)):
        for d in range(2):
            Wpack[:, :, 256 * (2 * w + d):256 * (2 * w + d + 1)] = \
                WT[:, 128 * d:128 * (d + 1), :]
    W1pack = np.zeros((NL, 128, 2048), np.float32)
    for d in range(2):
        W1pack[:, :, 1024 * d:1024 * (d + 1)] = W1T[:, 128 * d:128 * (d + 1), :]
    W2pack = np.zeros((NL, 128, 2048), np.float32)
    for f in range(8):
        W2pack[:, :, 256 * f:256 * (f + 1)] = W2T[:, 128 * f:128 * (f + 1), :]

    # packed biases [NL, 128, 24] f32:
    # cols 0-11: (bq,bk,bo,b2,ln1_s,ln2_s) x (d0,d1); 12-19: b1; 20-23: ln1_b,ln2_b
    bpack = np.zeros((NL, 128, 24), np.float32)
    bq = Wi["bqkv"][:, 0:D] * sc
    bk = Wi["bqkv"][:, D:2 * D]
    for col, arr in ((0, bq), (2, bk), (4, Wi["bo"]), (6, Wi["b2"]),
                     (8, Wi["ln1_s"]), (10, Wi["ln2_s"])):
        for d in range(2):
            bpack[:, :, col + d] = arr[:, 128 * d:128 * (d + 1)]
    for f in range(8):
        bpack[:, :, 12 + f] = Wi["b1"][:, 128 * f:128 * (f + 1)]
    for d in range(2):
        bpack[:, :, 20 + d] = Wi["ln1_b"][:, 128 * d:128 * (d + 1)]
        bpack[:, :, 22 + d] = Wi["ln2_b"][:, 128 * d:128 * (d + 1)]

    shared = dict(
        Wcat=Wcat.astype(BF),
        Wpack=Wpack.astype(BF),
        W1pack=W1pack.astype(BF),
        W2pack=W2pack.astype(BF),
        bpack=bpack,
    )
    WoutT = np.zeros((D, 16), np.float32)
    WoutT[:, :10] = Wi["W_out"].T
    bout = np.zeros((16, 1), np.float32)
    bout[:10, 0] = Wi["b_out"]
    shared["WoutT"] = WoutT.astype(BF)
    shared["bout"] = bout
    sel = np.zeros((128, 128), np.float32)
    for p in range(128):
        sel[32 * (p // 32), p] = 1.0
    shared["sel4"] = sel.astype(BF)

    in_maps, gidx_all = [], []
    for core in range(8):
        b, r = core // 4, core % 4
        gidx = np.concatenate([oc[b, r::4], C + oq[b, r::4]])
        gidx_all.append(gidx)
        trow = trow_all[b, r]
        # masks [128, 6*1024]: tile t -> [sec 6+t | sec t], allowed = tk <= tq
        masks = np.zeros((128, 6144), np.float32)
        for t in range(6):
            for half, s in ((0, 6 + t), (1, t)):
                tk = tc_sorted[b, 128 * s:128 * (s + 1)].astype(np.float32)
                masks[:, 1024 * t + 512 * half:1024 * t + 512 * (half + 1)] = (
                    tk[:, None] <= trow[None, :]).astype(np.float32)
        m = dict(shared)
        m["Xhat"] = np.ascontiguousarray(X[b, gidx].T).astype(BF)
        m["Xg"] = np.ascontiguousarray(X[b, oc[b]].T).astype(BF)
        m["masks"] = masks.astype(BF)
        m["vexpf"] = vexp[b]
        m["vexpb"] = vexp[b].astype(BF)
        in_maps.append(m)
    meta["gidx"] = gidx_all
    return in_maps, meta


def _layernorm(nc, ps, kpool, mybir, ones128, onesrow, eps, rin, s_ap, b_ap,
               tag, q0=0, c1=R, outres=None):
    """LN over the 256-dim partition axis (2 tiles), query cols [q0:c1].
    rin: 2 f32r tiles [128, R]. s_ap/b_ap: lists of [128,1] scale/bias APs.
    Returns (2 bf16 tiles, 2 f32 residual tiles) valid on [q0:c1]."""
    F32, F32R, BF16 = mybir.dt.float32, mybir.dt.float32r, mybir.dt.bfloat16
    ALU, ACTF = mybir.AluOpType, mybir.ActivationFunctionType
    pmu = ps.tile([1, R], F32, tag="pmm")
    for d in range(2):
        nc.tensor.matmul(pmu[:, q0:c1], ones128[:], rin[d][:, q0:c1],
                         start=(d == 0), stop=(d == 1))
    sq = [kpool.tile([128, R], F32R, tag=f"lnsq{d}", name=f"lnsq{d}") for d in range(2)]
    for d in range(2):
        nc.vector.scalar_tensor_tensor(
            sq[d][:, q0:c1], rin[d][:, q0:c1].bitcast(F32), 0.0,
            rin[d][:, q0:c1].bitcast(F32), ALU.add, ALU.mult)
    pms = ps.tile([1, R], F32, tag="pmm")
    for d in range(2):
        nc.tensor.matmul(pms[:, q0:c1], ones128[:], sq[d][:, q0:c1],
                         start=(d == 0), stop=(d == 1))
    mu = kpool.tile([1, R], F32R, tag=f"{tag}mu")
    nc.vector.tensor_scalar_mul(mu[:, q0:c1], pmu[:, q0:c1], 1.0 / 256.0)
    mu2 = kpool.tile([1, R], F32, tag=f"{tag}mu2")
    nc.vector.scalar_tensor_tensor(mu2[:, q0:c1], mu[:, q0:c1].bitcast(F32), 0.0,
                                   mu[:, q0:c1].bitcast(F32), ALU.add, ALU.mult)
    var = kpool.tile([1, R], F32, tag=f"{tag}var")
    nc.vector.scalar_tensor_tensor(var[:, q0:c1], pms[:, q0:c1], 1.0 / 256.0,
                                   mu2[:, q0:c1], ALU.mult, ALU.subtract)
    # 1/sqrt(var+eps) = exp(-0.5*ln(var+eps)); stays on the exp/ln ACT table
    lnv = kpool.tile([1, R], F32, tag=f"{tag}lnv")
    nc.scalar.activation(lnv[:, q0:c1], var[:, q0:c1], ACTF.Ln, bias=eps[:], scale=1.0)
    rs = kpool.tile([1, R], F32R, tag=f"{tag}rs")
    nc.scalar.activation(rs[:, q0:c1], lnv[:, q0:c1], ACTF.Exp, scale=-0.5)
    # broadcast mu/rs over partitions via PE outer-product (ones col x row)
    mu_b = ps.tile([128, R], F32, tag="pmm", name=f"{tag}mub")
    nc.tensor.matmul(mu_b[:, q0:c1], onesrow[:], mu[:, q0:c1], start=True, stop=True)
    rs_b = ps.tile([128, R], F32, tag="pmm", name=f"{tag}rsb")
    nc.tensor.matmul(rs_b[:, q0:c1], onesrow[:], rs[:, q0:c1], start=True, stop=True)
    del pmu, pms
    if outres is None:
        out = [kpool.tile([128, R], BF16, tag=f"{tag}o{d}", name=f"{tag}o{d}") for d in range(2)]
        res = [kpool.tile([128, R], F32, tag=f"{tag}r{d}", name=f"{tag}r{d}") for d in range(2)]
    else:
        out, res = outres
    for d in range(2):
        t1 = kpool.tile([128, R], F32, tag="lnt1")
        nc.vector.scalar_tensor_tensor(
            t1[:, q0:c1], rin[d][:, q0:c1].bitcast(F32), 0.0, mu_b[:, q0:c1],
            ALU.add, ALU.subtract)
        t2 = kpool.tile([128, R], F32, tag="lnt2")
        nc.vector.scalar_tensor_tensor(
            t2[:, q0:c1], t1[:, q0:c1], s_ap[d], rs_b[:, q0:c1],
            ALU.mult, ALU.mult)
        nc.vector.tensor_scalar_add(out[d][:, q0:c1], t2[:, q0:c1], b_ap[d])
        nc.vector.tensor_scalar_add(res[d][:, q0:c1], t2[:, q0:c1], b_ap[d])
    return out, res


def _build(meta):
    import sys
    if "/opt/trn_rl_repo" not in sys.path:
        sys.path.insert(0, "/opt/trn_rl_repo")
    import concourse.bacc as bacc
    import concourse.mybir as mybir
    import concourse.tile as tile

    F32, F32R, BF16 = mybir.dt.float32, mybir.dt.float32r, mybir.dt.bfloat16
    ALU, ACT = mybir.AluOpType, mybir.ActivationFunctionType
    qlo = [int(x) for x in meta["qlo"]]

    nc = bacc.Bacc("TRN2", target_bir_lowering=False, debug=False, num_devices=8)
    P = {}
    for n, shp, dt in [("Xhat", [52, R], BF16), ("Xg", [52, SK], BF16),
                       ("masks", [128, 6144], BF16),
                       ("vexpf", [128, NSEC], F32), ("vexpb", [128, NSEC], BF16),
                       ("sel4", [128, 128], BF16),
                       ("Wcat", [52, D], BF16),
                       ("Wpack", [NL, 128, 2048], BF16),
                       ("W1pack", [NL, 128, 2048], BF16),
                       ("W2pack", [NL, 128, 2048], BF16),
                       ("bpack", [NL, 128, 24], F32),
                       ("WoutT", [D, 16], BF16), ("bout", [16, 1], F32)]:
        P[n] = nc.declare_dram_parameter(n, shp, dt, isOutput=False)
    OUT = nc.declare_dram_parameter("OutT", [16, 128], F32, isOutput=True)

    with tile.TileContext(nc) as tc:
        with (
            tc.tile_pool(name="const", bufs=1) as cpool,
            tc.tile_pool(name="state", bufs=1) as spool,
            tc.tile_pool(name="w", bufs=2) as wpool,
            tc.tile_pool(name="work", bufs=1) as kpool,
            tc.tile_pool(name="pt", bufs=4) as ppool,
            tc.tile_pool(name="ps", bufs=2, space="PSUM") as ps,
            tc.tile_pool(name="psb", bufs=3, space="PSUM") as psb,
            tc.tile_pool(name="dram", bufs=1, space="DRAM") as dpool,
        ):
            # ---- static setup ----
            ones128 = cpool.tile([128, 1], F32R, tag="ones128")
            nc.vector.memset(ones128[:].bitcast(F32), 1.0)
            onesrow = cpool.tile([1, 128], F32R, tag="onesrow")
            nc.vector.memset(onesrow[:].bitcast(F32), 1.0)
            eps = cpool.tile([1, 1], F32, tag="eps")
            nc.vector.memset(eps[:], 1e-5)
            # head-selector matrix for the softmax-scale broadcast:
            # sel[c, p] = 1 iff c == 32*(p//32)
            sel = cpool.tile([128, 128], BF16, tag="sel")
            nc.gpsimd.dma_start(sel[:], P["sel4"][:])
            sums = spool.tile([128, R], F32, tag="sums", name="sums")
            nc.vector.memset(sums[:], 1.0)
            xhat = cpool.tile([52, R], BF16, tag="xhat")
            nc.gpsimd.dma_start(xhat[:], P["Xhat"][:])
            xg = cpool.tile([52, SK], BF16, tag="xg")
            nc.sync.dma_start(xg[:], P["Xg"][:])
            wcat = cpool.tile([52, D], BF16, tag="wcat")
            nc.gpsimd.dma_start(wcat[:], P["Wcat"][:])

            def load_weights(layer):
                wqkvo = wpool.tile([128, 2048], BF16, tag="wqkvo", name="wqkvo")
                nc.gpsimd.dma_start(wqkvo[:], P["Wpack"][layer])
                w1t = wpool.tile([128, 2048], BF16, tag="w1t", name="w1t")
                nc.gpsimd.dma_start(w1t[:], P["W1pack"][layer])
                w2t = wpool.tile([128, 2048], BF16, tag="w2t", name="w2t")
                nc.gpsimd.dma_start(w2t[:], P["W2pack"][layer])
                bp = wpool.tile([128, 24], F32, tag="bp", name="bp")
                nc.sync.dma_start(bp[:], P["bpack"][layer])
                return wqkvo, w1t, w2t, bp

            wts = load_weights(0)
            maskt = cpool.tile([128, 6144], BF16, tag="maskt")
            nc.sync.dma_start(maskt[:], P["masks"][:])
            vexpf = cpool.tile([128, NSEC], F32, tag="vexpf")
            nc.gpsimd.dma_start(vexpf[:], P["vexpf"][:])

            # K/Q tiles: per m-chunk, heads 4m..4m+2 at partition offsets
            # 0/32/64 of tile 2m, head 4m+3 at offset 0 of tile 2m+1 (the PE
            # only accepts base partitions 0/32/64). V holds 12 chunks of 33
            # cols per head (32 data + e^{at}).
            KHT = [spool.tile([128, SK], F32R, tag=f"KHT{i}", name=f"KHT{i}")
                   for i in range(4)]
            QHT = [spool.tile([128, R], F32R, tag=f"QHT{i}", name=f"QHT{i}")
                   for i in range(4)]

            def hslice(tiles, h, cols):
                ti, off = (2 * (h // 4), 32 * (h % 4)) if h % 4 < 3 \
                    else (2 * (h // 4) + 1, 0)
                return tiles[ti][off:off + 32, cols]
            VHT = spool.tile([128, NH * 33 * NSEC], BF16, tag="VHT", name="VHT")
            for h in range(NH):
                dst = VHT[:, 396 * h:396 * (h + 1)].rearrange(
                    "p (s c) -> p s c", c=33)[:, :, 32]
                nc.gpsimd.dma_start(dst, P["vexpb"][:])

            zown = [spool.tile([128, R], BF16, tag=f"zown{d}", name=f"zown{d}") for d in range(2)]
            zres = [spool.tile([128, R], F32, tag=f"zres{d}", name=f"zres{d}") for d in range(2)]

            # ---- embed ----
            for d in range(2):
                pe = ps.tile([128, R], F32, tag="pmm")
                nc.tensor.matmul(pe[:], wcat[:, 128 * d:128 * (d + 1)], xhat[:],
                                 start=True, stop=True)
                nc.vector.tensor_scalar_add(zown[d][:], pe[:], 0.0)
                nc.vector.tensor_scalar_add(zres[d][:], pe[:], 0.0)

            zg = [spool.tile([128, SK], BF16, tag=f"zg{d}", name=f"zg{d}") for d in range(2)]
            zs = [spool.tile([128, SK], BF16, tag=f"zs{d}", name=f"zs{d}") for d in range(2)]
            zb = dpool.tile([D, 384], BF16, tag="zb")
            zgat = dpool.tile([4 * D, 384], BF16, tag="zgat")
            groups = [[0, 1, 2, 3], [4, 5, 6, 7]]

            def issue_gather():
                for d in range(2):
                    nc.sync.dma_start(zb[128 * d:128 * (d + 1), :],
                                      zown[d][:, 0:384])
                nc.gpsimd.collective_compute(
                    "AllGather", ALU.bypass, replica_groups=groups,
                    ins=[zb.opt()], outs=[zgat.opt()])

            for layer in range(NL_RUN):
                last = (layer == NL - 1)
                q0 = Q0_LAST if last else 0

                wqkvo, w1t, w2t, bp = wts

                # ---- Q projection first: local input, overlaps the AllGather ----
                for m in range(2):
                    pq = ps.tile([128, R], F32, tag="pmm")
                    for d in range(2):
                        nc.tensor.matmul(
                            pq[:, q0:], wqkvo[:, 256 * d + 128 * m:256 * d + 128 * (m + 1)],
                            zown[d][:, q0:], start=(d == 0), stop=(d == 1))
                    nc.vector.tensor_scalar_add(QHT[2 * m][0:96, q0:],
                                                pq[0:96, q0:],
                                                bp[0:96, 0 + m:1 + m])
                    nc.vector.tensor_scalar_add(QHT[2 * m + 1][0:32, q0:],
                                                pq[96:128, q0:],
                                                bp[96:128, 0 + m:1 + m])
                if layer == 0:
                    # layer-0 "gather" is free: embed the host-shipped sorted
                    # ctx features directly (no collective, hides launch skew)
                    for d in range(2):
                        for g3 in range(3):
                            pz = ps.tile([128, R], F32, tag="pmm")
                            nc.tensor.matmul(
                                pz[:], wcat[:, 128 * d:128 * (d + 1)],
                                xg[:, 512 * g3:512 * (g3 + 1)],
                                start=True, stop=True)
                            nc.vector.tensor_scalar_add(
                                zs[d][:, 512 * g3:512 * (g3 + 1)], pz[:], 0.0)
                else:
                    # land the gathered ctx rows, then sort by time: sorted
                    # key i lives at zg col 384*(i%4) + i//4 -> zs col i
                    for d in range(2):
                        srcap = zgat[:].rearrange(
                            "(rk dd p) c -> p dd rk c", rk=4, dd=2)[:, d]
                        nc.gpsimd.dma_start(
                            zg[d][:].rearrange("p (rk c) -> p rk c", rk=4), srcap)
                    for d in range(2):
                        zso = zs[d][:].rearrange("p (a b) -> p a b", b=4)
                        zgi = zg[d][:].rearrange("p (b a) -> p a b", b=4)
                        nc.vector.tensor_scalar_add(zso, zgi, 0.0)

                # ---- K projection (sorted key order) ----
                for gp in range(3):
                    for m in range(2):
                        pk = ps.tile([128, R], F32, tag="pmm")
                        for d in range(2):
                            nc.tensor.matmul(
                                pk[:],
                                wqkvo[:, 512 + 256 * d + 128 * m:512 + 256 * d + 128 * (m + 1)],
                                zs[d][:, 512 * gp:512 * (gp + 1)],
                                start=(d == 0), stop=(d == 1))
                        nc.vector.tensor_scalar_add(
                            KHT[2 * m][0:96, 512 * gp:512 * (gp + 1)],
                            pk[0:96, :], bp[0:96, 2 + m:3 + m])
                        nc.vector.tensor_scalar_add(
                            KHT[2 * m + 1][0:32, 512 * gp:512 * (gp + 1)],
                            pk[96:128, :], bp[96:128, 2 + m:3 + m])

                # ---- V projection (rows = sorted key positions) ----
                for s in range(NSEC):
                    pv = psb.tile([128, 256], F32, tag="big", name=f"pv{s}")
                    for d in range(2):
                        nc.tensor.matmul(
                            pv[:], zs[d][:, 128 * s:128 * (s + 1)],
                            wqkvo[:, 1024 + 256 * d:1024 + 256 * (d + 1)],
                            start=(d == 0), stop=(d == 1))
                    outap = VHT[:].rearrange(
                        "p (h c) -> p h c", c=396)[:, :, 33 * s:33 * s + 32]
                    inap = pv[:].rearrange("p (h c) -> p h c", c=32)
                    nc.vector.tensor_scalar(outap, inap, vexpf[:, s:s + 1], None,
                                            ALU.mult)

                if layer + 1 < NL_RUN:
                    wts = load_weights(layer + 1)

                # ---- attention: per head, 6 two-section tiles, 1-tile
                # software pipeline (QK(t+1) ahead of PV(t)) ----
                at = [kpool.tile([128, R], BF16, tag=f"at{m}", name=f"at{m}") for m in range(2)]
                for h in range(NH):
                    m, h4 = h // 4, h % 4
                    pa = ps.tile([33, R], F32, tag="pmm", name=f"pa{h}")
                    tiles = []

                    def qk_tile(t):
                        shi, slo = 6 + t, t
                        e_hi = max(qlo[shi], q0)
                        e_lo = max(qlo[slo], q0)
                        sct = psb.tile([128, 1024], F32, tag="big")
                        nc.tensor.matmul(
                            sct[:, e_hi:512],
                            hslice(KHT, h, slice(128 * shi, 128 * (shi + 1))),
                            hslice(QHT, h, slice(e_hi, 512)),
                            start=True, stop=True)
                        nc.tensor.matmul(
                            sct[:, 512 + e_lo:1024],
                            hslice(KHT, h, slice(128 * slo, 128 * (slo + 1))),
                            hslice(QHT, h, slice(e_lo, 512)),
                            start=True, stop=True)
                        return sct, e_hi, e_lo

                    def drain_tile(t):
                        sct, e_hi, e_lo = tiles[t]
                        shi, slo = 6 + t, t
                        pt = ppool.tile([128, 1024], BF16, tag="ptile")
                        nc.scalar.activation(pt[:, e_hi:], sct[:, e_hi:], ACT.Exp)
                        nc.vector.tensor_tensor(
                            pt[:, e_hi:], pt[:, e_hi:],
                            maskt[:, 1024 * t + e_hi:1024 * (t + 1)], ALU.mult)
                        nc.tensor.matmul(
                            pa[:, e_lo:512], VHT[:, 396 * h + 33 * slo:396 * h + 33 * slo + 33],
                            pt[:, 512 + e_lo:1024], start=(t == 0), stop=False)
                        nc.tensor.matmul(
                            pa[:, e_hi:512], VHT[:, 396 * h + 33 * shi:396 * h + 33 * shi + 33],
                            pt[:, e_hi:512], start=False, stop=(t == 5))

                    for t in range(6):
                        tiles.append(qk_tile(t))
                        if t >= 2:
                            drain_tile(t - 2)
                    drain_tile(4)
                    drain_tile(5)
                    # row 32 of pa = sum_k e^{a t_k} p_k; park it at
                    # partition 32*h4 of the persistent sums tile
                    nc.vector.tensor_scalar_add(
                        sums[32 * h4:32 * h4 + 1, q0:], pa[32:33, q0:], 0.0)
                    nc.vector.tensor_scalar_add(
                        at[m][32 * h4:32 * (h4 + 1), q0:], pa[0:32, q0:], 0.0)
                    if h4 == 3:
                        lns = kpool.tile([128, R], F32, tag="lns", bufs=2)
                        nc.scalar.activation(lns[:, q0:], sums[:, q0:], ACT.Ln)
                        rcps = kpool.tile([128, R], BF16, tag="rcps", bufs=2)
                        nc.scalar.activation(rcps[:, q0:], lns[:, q0:],
                                             ACT.Exp, scale=-1.0)
                        rcp_b = ps.tile([128, R], F32, tag="pmm", name="rcpb")
                        nc.tensor.matmul(rcp_b[:, q0:], sel[:], rcps[:, q0:],
                                         start=True, stop=True)
                        nc.vector.tensor_tensor(at[m][:, q0:], at[m][:, q0:],
                                                rcp_b[:, q0:], ALU.mult)

                # ---- output proj + residual + LN1 ----
                r1 = [kpool.tile([128, R], F32R, tag=f"r1{d}", name=f"r1{d}") for d in range(2)]
                for m in range(2):
                    pp = ps.tile([128, R], F32, tag="pmm")
                    for d in range(2):
                        nc.tensor.matmul(
                            pp[:, q0:], wqkvo[:, 1536 + 256 * d + 128 * m:1536 + 256 * d + 128 * (m + 1)],
                            at[d][:, q0:], start=(d == 0), stop=(d == 1))
                    nc.vector.scalar_tensor_tensor(
                        r1[m][:, q0:], pp[:, q0:], bp[:, 4 + m:5 + m],
                        zres[m][:, q0:], ALU.add, ALU.add)
                lnz, lnres = _layernorm(nc, ps, kpool, mybir, ones128, onesrow,
                                        eps, r1, [bp[:, 8:9], bp[:, 9:10]],
                                        [bp[:, 20:21], bp[:, 21:22]], tag="ln1",
                                        q0=q0)

                # ---- FFN ----
                pf = [ps.tile([128, R], F32, tag="pmm", name=f"pf{m}") for m in range(2)]
                for f in range(8):
                    ph = psb.tile([128, R], F32, tag="big")
                    for d in range(2):
                        nc.tensor.matmul(
                            ph[:, q0:], w1t[:, 1024 * d + 128 * f:1024 * d + 128 * (f + 1)],
                            lnz[d][:, q0:], start=(d == 0), stop=(d == 1))
                    ht = ppool.tile([128, R], BF16, tag="htile")
                    nc.scalar.activation(ht[:, q0:], ph[:, q0:], ACT.Relu,
                                         bias=bp[:, 12 + f:13 + f], scale=1.0)
                    for m in range(2):
                        nc.tensor.matmul(
                            pf[m][:, q0:], w2t[:, 256 * f + 128 * m:256 * f + 128 * (m + 1)],
                            ht[:, q0:], start=(f == 0), stop=(f == 7))
                r2 = [kpool.tile([128, R], F32R, tag=f"r2{d}", name=f"r2{d}") for d in range(2)]
                for m in range(2):
                    nc.vector.scalar_tensor_tensor(
                        r2[m][:, q0:], pf[m][:, q0:], bp[:, 6 + m:7 + m],
                        lnres[m][:, q0:], ALU.add, ALU.add)
                if layer + 1 >= NL_RUN:
                    zown, zres = _layernorm(
                        nc, ps, kpool, mybir, ones128, onesrow, eps, r2,
                        [bp[:, 10:11], bp[:, 11:12]],
                        [bp[:, 22:23], bp[:, 23:24]], tag="ln2", q0=q0)
                else:
                    zown, zres = _layernorm(
                        nc, ps, kpool, mybir, ones128, onesrow, eps, r2,
                        [bp[:, 10:11], bp[:, 11:12]],
                        [bp[:, 22:23], bp[:, 23:24]], tag="ln2", q0=q0, c1=384)
                    issue_gather()
                    _layernorm(nc, ps, kpool, mybir, ones128, onesrow, eps, r2,
                               [bp[:, 10:11], bp[:, 11:12]],
                               [bp[:, 22:23], bp[:, 23:24]], tag="ln2",
                               q0=384, outres=(zown, zres))

            # ---- output head (qry cols only) ----
            wout = [cpool.tile([128, 16], BF16, tag=f"wout{d}", name=f"wout{d}") for d in range(2)]
            for d in range(2):
                nc.gpsimd.dma_start(wout[d][:], P["WoutT"][128 * d:128 * (d + 1), :])
            bo_t = cpool.tile([16, 1], F32, tag="bo_t")
            nc.sync.dma_start(bo_t[:], P["bout"][:])
            po = ps.tile([16, 128], F32, tag="pmm")
            for d in range(2):
                nc.tensor.matmul(po[:], wout[d][:],
                                 zown[d][:, 384:512], start=(d == 0), stop=(d == 1))
            oall = cpool.tile([16, 128], F32, tag="oall")
            nc.vector.tensor_scalar_add(oall[:], po[:], bo_t[:])
            nc.sync.dma_start(OUT[:], oall[:])

    nc.compile()
    return nc


def kernel(**inputs):
    import sys
    if "/opt/trn_rl_repo" not in sys.path:
        sys.path.insert(0, "/opt/trn_rl_repo")
    from concourse.bass_utils import run_bass_kernel_spmd

    in_maps, meta = _host_prep(inputs)
    nc = _build(meta)
    res = run_bass_kernel_spmd(nc, in_maps, list(range(8)))
    out = np.zeros((B, S, HOUT), np.float32)
    for c in range(8):
        b = c // 4
        o = res.results[c]["OutT"]          # [16, 128]
        out[b, meta["gidx"][c][384:]] = o[:HOUT].T
    return np.ascontiguousarray(out[:, C:, :]).astype(np.float32)
